# revision 30
# baseline (speedup 1.0000x reference)
"""Trainium2 Bass kernel for nn_MaxPoolAggregator (GNN max-pool message passing).

reference:
    norm = x @ W1                       # [N, D]
    pooled[d] = max over edges (s,d) of norm[s]   (0 for dsts with no edges)
    out = concat([x, pooled], axis=1)   # [N, 2D]

Strategy (8 NeuronCores, dst-sharded, zero on-device gather):
  - Destination nodes sharded: core k owns dsts [k*6250, (k+1)*6250).
  - Host-side, each core's edge list is turned into a dst-major "slot tape"
    sigma: for each dst (degree-sorted desc), its source node ids occupy K
    consecutive slots (K = padded max degree of the chunk), padding slots
    point at a dummy column whose norm is ~-1000 (pre-solved so that
    W1^T v = -1000*ones); zero-degree dsts point at a zero column.
  - The host materializes xS = x[:, sigma] in bf16 ([128 feat, S slots]),
    so the device just streams xS, computes norm^T = W1^T @ xS on the PE
    (slots land in the PSUM free dim already grouped by dst), and reduces
    each dst's K-slot window with a short max tree.  No gathers, no index
    tables, no norm round-trip through DRAM.
  - Reduce work is split across DVE and ACT per chunk (STRAT_PATTERN):
    'b' = ACT copies PSUM->SBUF bf16 then DVE runs a pairwise max tree at
    the 2x DVE rate; 'r' = one DVE tensor_reduce straight from PSUM.
    (GPSIMD tensor ops and InstPool do not lower in this compile pipeline;
    DVE tensor_tensor may read at most one PSUM operand.)
  - Output pooled^T [128 feat, 6250 dst-ranks] in bf16; host unpermutes the
    degree-sort and concatenates with x.
"""

import numpy as np

N_NODES = 50000
D = 128
CORES = 8
NB = N_NODES // CORES          # 6250 dsts per core
CHUNK = 2048                   # slots per PSUM chunk (= 4 banks fp32)
SLAB = 2048                    # slots per DMA slab (= 1 chunk)
NEG_IDX = N_NODES              # dummy column -> norm ~= -1000
ZERO_IDX = N_NODES + 1         # dummy column -> norm == 0
NEG_M = 1000.0

# per-chunk reduce-engine strategy, cycled by chunk index:
#   b: ACT copy (PSUM->SBUF bf16) + DVE bf16 max tree
#   r: single DVE tensor_reduce straight from PSUM
#   h/i: ACT copies half, DVE handles the rest (see emit_reduce)
# "bbbr" balances ACT (copies) against DVE (trees + direct reduces).
STRAT_PATTERN = "bbbr"
PSUM_BUFS = 2

_CACHE = {}


def _chunk_plan(dmax):
    """Shared chunk structure from the elementwise-max degree profile.

    Returns list of (r0, C, K): C dst-ranks starting at r0, K slots each,
    occupying one CHUNK-slot window (padded to CHUNK).
    """
    chunks = []
    r = 0
    while r < NB:
        K = min(max(1, int(dmax[r])), CHUNK)
        C = max(1, min(CHUNK // K, NB - r))
        chunks.append((r, C, K))
        r += C
    return chunks


def _prep(edge_index):
    """Per-core degree sort + slot tapes. Returns (chunks, per_core)."""
    src = np.asarray(edge_index[0]).astype(np.int64)
    dst = np.asarray(edge_index[1]).astype(np.int64)
    cores = []
    for k in range(CORES):
        m = (dst >= k * NB) & (dst < (k + 1) * NB)
        d = dst[m] - k * NB
        deg = np.bincount(d, minlength=NB)
        order = np.argsort(-deg, kind="stable")
        rank = np.empty(NB, np.int64)
        rank[order] = np.arange(NB)
        cores.append(dict(deg=deg, order=order, rank=rank, dd=d, vv=src[m]))

    degs_sorted = np.stack([c["deg"][c["order"]] for c in cores])
    dmax = degs_sorted.max(axis=0)
    chunks = _chunk_plan(dmax)
    S = len(chunks) * CHUNK

    # rank -> slot base of its K-window
    rbase = np.empty(NB, np.int64)
    for i, (r0, C, K) in enumerate(chunks):
        rbase[r0:r0 + C] = i * CHUNK + np.arange(C) * K

    per_core = []
    for c in cores:
        sigma = np.full(S, NEG_IDX, np.int64)
        dd, vv, rank, deg = c["dd"], c["vv"], c["rank"], c["deg"]
        if dd.size:
            r_e = rank[dd]
            es = np.argsort(r_e, kind="stable")
            rs, vs = r_e[es], vv[es]
            cnt_sorted = deg[c["order"]]
            starts = np.concatenate([[0], np.cumsum(cnt_sorted)[:-1]])
            j = np.arange(rs.size) - starts[rs]
            sigma[rbase[rs] + j] = vs
        zr = rank[deg == 0]
        if zr.size:
            sigma[rbase[zr]] = ZERO_IDX
        per_core.append(dict(sigma=sigma, order=c["order"]))
    return chunks, per_core


def _build_nc(chunks):
    import concourse.bacc as bacc
    import concourse.mybir as mybir
    import concourse.tile as tile

    f32 = mybir.dt.float32
    bf16 = mybir.dt.bfloat16
    Copy = mybir.ActivationFunctionType.Copy
    S = len(chunks) * CHUNK

    nc = bacc.Bacc("TRN2", target_bir_lowering=False, debug=False)
    xS_d = nc.dram_tensor("xS", [D, S], bf16, kind="ExternalInput")
    w1_d = nc.dram_tensor("W1b", [D, D], bf16, kind="ExternalInput")
    out_d = nc.dram_tensor("out", [D, NB], bf16, kind="ExternalOutput")

    with tile.TileContext(nc) as tc:
        with (
            tc.tile_pool(name="const", bufs=1) as cpool,
            tc.tile_pool(name="x", bufs=3) as xpool,
            tc.tile_pool(name="psum", bufs=PSUM_BUFS, space="PSUM") as ppool,
            tc.tile_pool(name="stage", bufs=2) as spool,
            tc.tile_pool(name="acc", bufs=1) as apool,
        ):
            w1t = cpool.tile([D, D], bf16)
            nc.sync.dma_start(out=w1t[:], in_=w1_d[:])
            pooled = apool.tile([D, NB], bf16)

            def tree(cur_tile, C, Kc, r0, depth):
                """DVE bf16 SBUF max tree: [C,Kc] -> pooled[:, r0:r0+C]."""
                eng = nc.vector
                while Kc > 1:
                    Kn = (Kc + 1) // 2
                    cur3 = cur_tile[:, :C * Kc].rearrange(
                        "p (c k) -> p c k", k=Kc)
                    if Kn == 1:
                        o3 = pooled[:, r0:r0 + C].rearrange(
                            "p (c k) -> p c k", k=1)
                    else:
                        nxt = spool.tile([D, 3072], bf16, tag=f"st{depth}")
                        o3 = nxt[:, :C * Kn].rearrange("p (c k) -> p c k", k=Kn)
                    eng.tensor_max(
                        out=o3, in0=cur3[:, :, 0:Kn], in1=cur3[:, :, Kc - Kn:Kc])
                    if Kn > 1:
                        cur_tile = nxt
                    Kc = Kn
                    depth += 1

            def emit_reduce(strat, ps, C, K, r0):
                if K == 1:
                    # degree-1 window: plain downcast copy into pooled
                    nc.scalar.activation(
                        out=pooled[:, r0:r0 + C], in_=ps[:, :C], func=Copy)
                    return
                ps3 = ps[:, :C * K].rearrange("p (c k) -> p c k", k=K)
                if strat == "r":
                    # single DVE reduce straight from PSUM
                    nc.vector.tensor_reduce(
                        out=pooled[:, r0:r0 + C], in_=ps3,
                        axis=mybir.AxisListType.X, op=mybir.AluOpType.max)
                elif strat == "b":          # ACT full copy + DVE tree
                    cp = spool.tile([D, CHUNK], bf16, tag="cpb")
                    nc.scalar.activation(
                        out=cp[:, :C * K], in_=ps[:, :C * K], func=Copy)
                    tree(cp, C, K, r0, 0)
                elif strat == "i":
                    # stall-free split: DVE reduces the head half straight
                    # from PSUM while ACT copies the tail half (independent
                    # readers -> PSUM frees after the slower of the two),
                    # then DVE trees the tail and combines.
                    K1 = (K + 1) // 2
                    K2 = K - K1
                    stA = spool.tile([D, 1024], bf16, tag="iA")
                    nc.vector.tensor_reduce(
                        out=stA[:, :C], in_=ps3[:, :, 0:K1],
                        axis=mybir.AxisListType.X, op=mybir.AluOpType.max)
                    cp = spool.tile([D, 1536], bf16, tag="iC")
                    cp3 = cp[:, :C * K2].rearrange("p (c k) -> p c k", k=K2)
                    nc.scalar.activation(
                        out=cp3, in_=ps3[:, :, K1:K], func=Copy)
                    if K2 == 1:
                        stB = cp
                    else:
                        stB = spool.tile([D, 1024], bf16, tag="iB")
                        treeK = K2
                        cur = cp
                        while treeK > 2:
                            Kn = (treeK + 1) // 2
                            nxt = spool.tile([D, 1536], bf16,
                                             tag=f"it{treeK}")
                            c3 = cur[:, :C * treeK].rearrange(
                                "p (c k) -> p c k", k=treeK)
                            nc.vector.tensor_max(
                                out=nxt[:, :C * Kn].rearrange(
                                    "p (c k) -> p c k", k=Kn),
                                in0=c3[:, :, 0:Kn],
                                in1=c3[:, :, treeK - Kn:treeK])
                            cur, treeK = nxt, Kn
                        c3 = cur[:, :C * treeK].rearrange(
                            "p (c k) -> p c k", k=treeK)
                        nc.vector.tensor_max(
                            out=stB[:, :C].rearrange("p (c k) -> p c k", k=1),
                            in0=c3[:, :, 0:1], in1=c3[:, :, treeK - 1:treeK])
                    nc.vector.tensor_max(
                        out=pooled[:, r0:r0 + C], in0=stA[:, :C],
                        in1=stB[:, :C])
                elif strat == "h":
                    # ACT copies the tail half, DVE maxes PSUM head vs SBUF
                    # tail (one PSUM operand only), then DVE tree
                    K1 = (K + 1) // 2
                    cp = spool.tile([D, 1536], bf16, tag="cph")
                    nc.scalar.activation(
                        out=cp[:, :C * K1].rearrange("p (c k) -> p c k", k=K1),
                        in_=ps3[:, :, K - K1:K], func=Copy)
                    if K1 == 1:
                        o3 = pooled[:, r0:r0 + C].rearrange(
                            "p (c k) -> p c k", k=1)
                        st = None
                    else:
                        st = spool.tile([D, 1536], bf16, tag="s1")
                        o3 = st[:, :C * K1].rearrange("p (c k) -> p c k", k=K1)
                    nc.vector.tensor_max(
                        out=o3, in0=ps3[:, :, 0:K1],
                        in1=cp[:, :C * K1].rearrange("p (c k) -> p c k", k=K1))
                    if st is not None:
                        tree(st, C, K1, r0, 1)
                else:
                    raise ValueError(strat)

            nchunks = len(chunks)
            pat = STRAT_PATTERN
            cps = SLAB // CHUNK            # chunks per slab
            nslabs = (nchunks + cps - 1) // cps
            flushed = 0
            for s in range(nslabs):
                w = min(SLAB, S - s * SLAB)
                xt = xpool.tile([D, SLAB], bf16, tag="xt")
                nc.sync.dma_start(out=xt[:, :w], in_=xS_d[:, s * SLAB:s * SLAB + w])
                tok = pat[s % len(pat)]
                ids = [ci for ci in range(s * cps, min(s * cps + cps, nchunks))]
                KA = chunks[ids[0]][2]
                KB = chunks[ids[-1]][2] if len(ids) == 2 else -1
                pair = (len(ids) == 2 and tok in ("B", "H") and KA > 1
                        and KB > 1
                        and ((tok == "B" and KA == KB)
                             or (tok == "H" and (KA + 1) // 2 == (KB + 1) // 2)))
                if pair:
                    cp2 = spool.tile([D, 2 * CHUNK], bf16, tag="cp" + tok)
                    if tok == "H":
                        st2 = spool.tile([D, 3072], bf16, tag="stH")
                    off = 0
                pooled_off = chunks[ids[0]][0]
                for h, ci in enumerate(ids):
                    r0, C, K = chunks[ci]
                    ps = ppool.tile([D, CHUNK], f32, tag="ps")
                    for mo in range(0, CHUNK, 512):
                        nc.tensor.matmul(
                            out=ps[:, mo:mo + 512],
                            lhsT=w1t[:],
                            rhs=xt[:, h * CHUNK + mo:h * CHUNK + mo + 512],
                            start=True,
                            stop=True,
                        )
                    if not pair:
                        t = tok.lower() if tok in "BH" else tok
                        emit_reduce(t, ps, C, K, r0)
                        continue
                    # paired path: stage this chunk eagerly, tree at the end
                    if tok == "B":
                        nc.scalar.activation(
                            out=cp2[:, off:off + C * K], in_=ps[:, :C * K],
                            func=Copy)
                        off += C * K
                    else:
                        K1 = (K + 1) // 2
                        ps3 = ps[:, :C * K].rearrange("p (c k) -> p c k", k=K)
                        cpw = cp2[:, off:off + C * K1].rearrange(
                            "p (c k) -> p c k", k=K1)
                        nc.scalar.activation(
                            out=cpw, in_=ps3[:, :, K - K1:K], func=Copy)
                        if K1 == 1:
                            o3 = pooled[:, r0:r0 + C].rearrange(
                                "p (c k) -> p c k", k=1)
                        else:
                            o3 = st2[:, off:off + C * K1].rearrange(
                                "p (c k) -> p c k", k=K1)
                        nc.vector.tensor_max(
                            out=o3, in0=ps3[:, :, 0:K1], in1=cpw)
                        off += C * K1
                if pair:
                    C2 = chunks[ids[0]][1] + chunks[ids[1]][1]
                    if tok == "B":
                        tree(cp2, C2, KA, pooled_off, 0)
                    elif (KA + 1) // 2 > 1:
                        tree(st2, C2, (KA + 1) // 2, pooled_off, 1)

            nc.sync.dma_start(
                out=out_d[:, flushed:NB], in_=pooled[:, flushed:NB])
    nc.compile()
    return nc


def _get_program(chunks):
    key = tuple(chunks)
    if key not in _CACHE:
        _CACHE[key] = _build_nc(chunks)
    return _CACHE[key]


def kernel(x, W1, edge_index, _return_extra=False):
    import ml_dtypes
    from concourse.bass_utils import run_bass_kernel_spmd

    bf16 = ml_dtypes.bfloat16
    x = np.asarray(x, np.float32)
    W1 = np.asarray(W1, np.float32)
    chunks, per_core = _prep(edge_index)
    nc = _get_program(chunks)

    W1b = W1.astype(bf16)
    # dummy column v with W1b^T v = -NEG_M * ones  (padding slots)
    vneg = np.linalg.solve(W1b.astype(np.float64).T,
                           np.full(D, -NEG_M, np.float64))
    xbigT = np.empty((D, N_NODES + 2), bf16)
    xbigT[:, :N_NODES] = x.astype(bf16).T
    xbigT[:, N_NODES] = vneg.astype(bf16)
    xbigT[:, N_NODES + 1] = 0

    in_maps = []
    for k in range(CORES):
        in_maps.append({
            "xS": np.ascontiguousarray(xbigT[:, per_core[k]["sigma"]]),
            "W1b": np.ascontiguousarray(W1b),
        })
    res = run_bass_kernel_spmd(nc, in_maps, list(range(CORES)))

    pooled = np.empty((N_NODES, D), np.float32)
    for k in range(CORES):
        out_k = np.asarray(res.results[k]["out"]).astype(np.float32)  # [D, NB]
        blk = np.empty((NB, D), np.float32)
        blk[per_core[k]["order"]] = out_k.T
        pooled[k * NB:(k + 1) * NB] = blk
    full = np.concatenate([x, pooled], axis=1)
    if _return_extra:
        return full, res
    return full


# revision 39
# speedup vs baseline: 1.0006x; 1.0006x over previous
"""Trainium2 Bass kernel for nn_MaxPoolAggregator (GNN max-pool message passing).

reference:
    norm = x @ W1                       # [N, D]
    pooled[d] = max over edges (s,d) of norm[s]   (0 for dsts with no edges)
    out = concat([x, pooled], axis=1)   # [N, 2D]

Strategy (8 NeuronCores, dst-sharded, zero on-device gather):
  - Destination nodes sharded: core k owns dsts [k*6250, (k+1)*6250).
  - Host-side, each core's edge list is turned into a dst-major "slot tape"
    sigma: for each dst (degree-sorted desc), its source node ids occupy K
    consecutive slots (K = padded max degree of the chunk), padding slots
    point at a dummy column whose norm is ~-1000 (pre-solved so that
    W1^T v = -1000*ones); zero-degree dsts point at a zero column.
  - The host materializes xS = x[:, sigma] in bf16 ([128 feat, S slots]),
    so the device just streams xS, computes norm^T = W1^T @ xS on the PE
    (slots land in the PSUM free dim already grouped by dst), and reduces
    each dst's K-slot window with a short max tree.  No gathers, no index
    tables, no norm round-trip through DRAM.
  - Reduce work is split across DVE and ACT per chunk (STRAT_PATTERN):
    'b' = ACT copies PSUM->SBUF bf16 then DVE runs a pairwise max tree at
    the 2x DVE rate; 'r' = one DVE tensor_reduce straight from PSUM.
    (GPSIMD tensor ops and InstPool do not lower in this compile pipeline;
    DVE tensor_tensor may read at most one PSUM operand.)
  - Output pooled^T [128 feat, 6250 dst-ranks] in bf16; host unpermutes the
    degree-sort and concatenates with x.
"""

import numpy as np

N_NODES = 50000
D = 128
CORES = 8
NB = N_NODES // CORES          # 6250 dsts per core
CHUNK = 2048                   # slots per PSUM chunk (= 4 banks fp32)
SLAB = 2048                    # slots per DMA slab (= 1 chunk)
NEG_IDX = N_NODES              # dummy column -> norm ~= -1000
ZERO_IDX = N_NODES + 1         # dummy column -> norm == 0
NEG_M = 1000.0

# per-chunk reduce-engine strategy, cycled by chunk index:
#   b: ACT copy (PSUM->SBUF bf16) + DVE bf16 max tree
#   r: single DVE tensor_reduce straight from PSUM
#   h/i: ACT copies half, DVE handles the rest (see emit_reduce)
# "bbbr" balances ACT (copies) against DVE (trees + direct reduces).
STRAT_PATTERN = "bbbr"
PSUM_BUFS = 2
SPLIT_FRAC = 0.30               # dst fraction DVE direct-reduces in strategy "s"
TREE_TAIL_FOLD = 0              # tail-fold disabled: tiny tree levels beat one 1x-rate reduce

_CACHE = {}


def _chunk_plan(dmax):
    """Shared chunk structure from the elementwise-max degree profile.

    Returns list of (r0, C, K): C dst-ranks starting at r0, K slots each,
    occupying one CHUNK-slot window (padded to CHUNK).
    """
    chunks = []
    r = 0
    off = 0
    while r < NB:
        K = min(max(1, int(dmax[r])), CHUNK)
        C = max(1, min(CHUNK // K, NB - r))
        chunks.append((r, C, K, off))
        off += C * K
        r += C
    return chunks


def _prep(edge_index):
    """Per-core degree sort + slot tapes. Returns (chunks, per_core)."""
    src = np.asarray(edge_index[0]).astype(np.int64)
    dst = np.asarray(edge_index[1]).astype(np.int64)
    cores = []
    for k in range(CORES):
        m = (dst >= k * NB) & (dst < (k + 1) * NB)
        d = dst[m] - k * NB
        deg = np.bincount(d, minlength=NB)
        order = np.argsort(-deg, kind="stable")
        rank = np.empty(NB, np.int64)
        rank[order] = np.arange(NB)
        cores.append(dict(deg=deg, order=order, rank=rank, dd=d, vv=src[m]))

    degs_sorted = np.stack([c["deg"][c["order"]] for c in cores])
    dmax = degs_sorted.max(axis=0)
    chunks = _chunk_plan(dmax)
    r0, C, K, off = chunks[-1]
    S = off + C * K

    # rank -> slot base of its K-window (tightly packed tape)
    rbase = np.empty(NB, np.int64)
    for r0, C, K, off in chunks:
        rbase[r0:r0 + C] = off + np.arange(C) * K

    per_core = []
    for c in cores:
        sigma = np.full(S, NEG_IDX, np.int64)
        dd, vv, rank, deg = c["dd"], c["vv"], c["rank"], c["deg"]
        if dd.size:
            r_e = rank[dd]
            es = np.argsort(r_e, kind="stable")
            rs, vs = r_e[es], vv[es]
            cnt_sorted = deg[c["order"]]
            starts = np.concatenate([[0], np.cumsum(cnt_sorted)[:-1]])
            j = np.arange(rs.size) - starts[rs]
            sigma[rbase[rs] + j] = vs
        zr = rank[deg == 0]
        if zr.size:
            sigma[rbase[zr]] = ZERO_IDX
        per_core.append(dict(sigma=sigma, order=c["order"]))
    return chunks, per_core


def _build_nc(chunks):
    import concourse.bacc as bacc
    import concourse.mybir as mybir
    import concourse.tile as tile

    f32 = mybir.dt.float32
    bf16 = mybir.dt.bfloat16
    Copy = mybir.ActivationFunctionType.Copy
    S = chunks[-1][3] + chunks[-1][1] * chunks[-1][2]

    nc = bacc.Bacc("TRN2", target_bir_lowering=False, debug=False)
    xS_d = nc.dram_tensor("xS", [D, S], bf16, kind="ExternalInput")
    w1_d = nc.dram_tensor("W1b", [D, D], bf16, kind="ExternalInput")
    out_d = nc.dram_tensor("out", [D, NB], bf16, kind="ExternalOutput")

    with tile.TileContext(nc) as tc:
        with (
            tc.tile_pool(name="const", bufs=1) as cpool,
            tc.tile_pool(name="x", bufs=3) as xpool,
            tc.tile_pool(name="psum", bufs=PSUM_BUFS, space="PSUM") as ppool,
            tc.tile_pool(name="stage", bufs=2) as spool,
            tc.tile_pool(name="acc", bufs=1) as apool,
        ):
            w1t = cpool.tile([D, D], bf16)
            nc.sync.dma_start(out=w1t[:], in_=w1_d[:])
            pooled = apool.tile([D, NB], bf16)

            def tree(cur_tile, C, Kc, r0, depth):
                """DVE bf16 SBUF max tree: [C,Kc] -> pooled[:, r0:r0+C]."""
                eng = nc.vector
                while Kc > 1:
                    if Kc > 2 and C * Kc <= TREE_TAIL_FOLD:
                        # small residual: one strided reduce beats more
                        # tiny tree levels (per-op overhead dominates)
                        cur3 = cur_tile[:, :C * Kc].rearrange(
                            "p (c k) -> p c k", k=Kc)
                        eng.tensor_reduce(
                            out=pooled[:, r0:r0 + C], in_=cur3,
                            axis=mybir.AxisListType.X, op=mybir.AluOpType.max)
                        return
                    Kn = (Kc + 1) // 2
                    cur3 = cur_tile[:, :C * Kc].rearrange(
                        "p (c k) -> p c k", k=Kc)
                    if Kn == 1:
                        o3 = pooled[:, r0:r0 + C].rearrange(
                            "p (c k) -> p c k", k=1)
                    else:
                        nxt = spool.tile([D, 3072], bf16, tag=f"st{depth}")
                        o3 = nxt[:, :C * Kn].rearrange("p (c k) -> p c k", k=Kn)
                    eng.tensor_max(
                        out=o3, in0=cur3[:, :, 0:Kn], in1=cur3[:, :, Kc - Kn:Kc])
                    if Kn > 1:
                        cur_tile = nxt
                    Kc = Kn
                    depth += 1

            def emit_reduce(strat, ps, C, K, r0):
                if K == 1:
                    # degree-1 window: plain downcast copy into pooled
                    nc.scalar.activation(
                        out=pooled[:, r0:r0 + C], in_=ps[:, :C], func=Copy)
                    return
                ps3 = ps[:, :C * K].rearrange("p (c k) -> p c k", k=K)
                if strat == "r":
                    # single DVE reduce straight from PSUM
                    nc.vector.tensor_reduce(
                        out=pooled[:, r0:r0 + C], in_=ps3,
                        axis=mybir.AxisListType.X, op=mybir.AluOpType.max)
                elif strat == "s":
                    # per-chunk dst-range split: DVE direct-reduces the last
                    # SPLIT_FRAC of the dsts straight from PSUM while ACT
                    # copies the rest (independent readers, no PSUM stall);
                    # DVE then trees the copied part at the 2x bf16 rate.
                    cs = C - max(1, min(C - 1, int(round(C * SPLIT_FRAC))))
                    nc.vector.tensor_reduce(
                        out=pooled[:, r0 + cs:r0 + C], in_=ps3[:, cs:C, :],
                        axis=mybir.AxisListType.X, op=mybir.AluOpType.max)
                    cp = spool.tile([D, CHUNK], bf16, tag="cps")
                    nc.scalar.activation(
                        out=cp[:, :cs * K], in_=ps[:, :cs * K], func=Copy)
                    tree(cp, cs, K, r0, 0)
                elif strat == "b":          # ACT full copy + DVE tree
                    cp = spool.tile([D, CHUNK], bf16, tag="cpb")
                    nc.scalar.activation(
                        out=cp[:, :C * K], in_=ps[:, :C * K], func=Copy)
                    tree(cp, C, K, r0, 0)
                elif strat == "i":
                    # stall-free split: DVE reduces the head half straight
                    # from PSUM while ACT copies the tail half (independent
                    # readers -> PSUM frees after the slower of the two),
                    # then DVE trees the tail and combines.
                    K1 = (K + 1) // 2
                    K2 = K - K1
                    stA = spool.tile([D, 1024], bf16, tag="iA")
                    nc.vector.tensor_reduce(
                        out=stA[:, :C], in_=ps3[:, :, 0:K1],
                        axis=mybir.AxisListType.X, op=mybir.AluOpType.max)
                    cp = spool.tile([D, 1536], bf16, tag="iC")
                    cp3 = cp[:, :C * K2].rearrange("p (c k) -> p c k", k=K2)
                    nc.scalar.activation(
                        out=cp3, in_=ps3[:, :, K1:K], func=Copy)
                    if K2 == 1:
                        stB = cp
                    else:
                        stB = spool.tile([D, 1024], bf16, tag="iB")
                        treeK = K2
                        cur = cp
                        while treeK > 2:
                            Kn = (treeK + 1) // 2
                            nxt = spool.tile([D, 1536], bf16,
                                             tag=f"it{treeK}")
                            c3 = cur[:, :C * treeK].rearrange(
                                "p (c k) -> p c k", k=treeK)
                            nc.vector.tensor_max(
                                out=nxt[:, :C * Kn].rearrange(
                                    "p (c k) -> p c k", k=Kn),
                                in0=c3[:, :, 0:Kn],
                                in1=c3[:, :, treeK - Kn:treeK])
                            cur, treeK = nxt, Kn
                        c3 = cur[:, :C * treeK].rearrange(
                            "p (c k) -> p c k", k=treeK)
                        nc.vector.tensor_max(
                            out=stB[:, :C].rearrange("p (c k) -> p c k", k=1),
                            in0=c3[:, :, 0:1], in1=c3[:, :, treeK - 1:treeK])
                    nc.vector.tensor_max(
                        out=pooled[:, r0:r0 + C], in0=stA[:, :C],
                        in1=stB[:, :C])
                elif strat == "h":
                    # ACT copies the tail half, DVE maxes PSUM head vs SBUF
                    # tail (one PSUM operand only), then DVE tree
                    K1 = (K + 1) // 2
                    cp = spool.tile([D, 1536], bf16, tag="cph")
                    nc.scalar.activation(
                        out=cp[:, :C * K1].rearrange("p (c k) -> p c k", k=K1),
                        in_=ps3[:, :, K - K1:K], func=Copy)
                    if K1 == 1:
                        o3 = pooled[:, r0:r0 + C].rearrange(
                            "p (c k) -> p c k", k=1)
                        st = None
                    else:
                        st = spool.tile([D, 1536], bf16, tag="s1")
                        o3 = st[:, :C * K1].rearrange("p (c k) -> p c k", k=K1)
                    nc.vector.tensor_max(
                        out=o3, in0=ps3[:, :, 0:K1],
                        in1=cp[:, :C * K1].rearrange("p (c k) -> p c k", k=K1))
                    if st is not None:
                        tree(st, C, K1, r0, 1)
                else:
                    raise ValueError(strat)

            pat = STRAT_PATTERN
            for ci, (r0, C, K, off) in enumerate(chunks):
                w = C * K
                xt = xpool.tile([D, CHUNK], bf16, tag="xt")
                nc.sync.dma_start(out=xt[:, :w], in_=xS_d[:, off:off + w])
                ps = ppool.tile([D, CHUNK], f32, tag="ps")
                for mo in range(0, w, 512):
                    mw = min(512, w - mo)
                    nc.tensor.matmul(
                        out=ps[:, mo:mo + mw],
                        lhsT=w1t[:],
                        rhs=xt[:, mo:mo + mw],
                        start=True,
                        stop=True,
                    )
                emit_reduce(pat[ci % len(pat)], ps, C, K, r0)

            nc.sync.dma_start(out=out_d[:], in_=pooled[:])
    nc.compile()
    return nc


def _get_program(chunks):
    key = tuple(chunks)
    if key not in _CACHE:
        _CACHE[key] = _build_nc(chunks)
    return _CACHE[key]


def kernel(x, W1, edge_index, _return_extra=False):
    import ml_dtypes
    from concourse.bass_utils import run_bass_kernel_spmd

    bf16 = ml_dtypes.bfloat16
    x = np.asarray(x, np.float32)
    W1 = np.asarray(W1, np.float32)
    chunks, per_core = _prep(edge_index)
    nc = _get_program(chunks)

    W1b = W1.astype(bf16)
    # dummy column v with W1b^T v = -NEG_M * ones  (padding slots)
    vneg = np.linalg.solve(W1b.astype(np.float64).T,
                           np.full(D, -NEG_M, np.float64))
    xbigT = np.empty((D, N_NODES + 2), bf16)
    xbigT[:, :N_NODES] = x.astype(bf16).T
    xbigT[:, N_NODES] = vneg.astype(bf16)
    xbigT[:, N_NODES + 1] = 0

    in_maps = []
    for k in range(CORES):
        in_maps.append({
            "xS": np.ascontiguousarray(xbigT[:, per_core[k]["sigma"]]),
            "W1b": np.ascontiguousarray(W1b),
        })
    res = run_bass_kernel_spmd(nc, in_maps, list(range(CORES)))

    pooled = np.empty((N_NODES, D), np.float32)
    for k in range(CORES):
        out_k = np.asarray(res.results[k]["out"]).astype(np.float32)  # [D, NB]
        blk = np.empty((NB, D), np.float32)
        blk[per_core[k]["order"]] = out_k.T
        pooled[k * NB:(k + 1) * NB] = blk
    full = np.concatenate([x, pooled], axis=1)
    if _return_extra:
        return full, res
    return full
